# revision 1
# baseline (speedup 1.0000x reference)
"""Trainium2 Bass kernel for channel-wise weighted reduction + capped relu.

Computes out[b, s] = capped_relu(sum_c x[b,c,s] * W[c,s] + bias[s]) for
x [64, 256, 4096] f32, W [256, 4096] f32, bias [4096] f32.

Sharding: data-parallel over batch across 8 NeuronCores (8 batches/core),
weights + bias replicated. No cross-core communication.

Per-core pipeline:
  - DMA x[b] as one SBUF tile [128ch, 2*4096] (two 2 MiB transfers).
  - DVE: y = x * W elementwise (in-place), one [128, 4096] op per c-half.
  - PE:  channel reduction as matmul with ones[128,1] STATIONARY (loaded
    once, 1 column) and the products MOVING: out row psum[b, chunk] =
    ones.T @ y_chunk. fp32 moving rows cost 4 cyc/row; for FOLD_BATCHES
    of the 8 batches the two c-halves are pre-summed on DVE (one extra
    [128,4096] add) which halves that batch's PE stream — the knob
    balances DVE vs PE occupancy.
  - Epilogue on [8, 4096]: tb = psum + bias ; mask = is_le(max(tb,0),1) ;
    o = max(tb,0)*mask ; direct row-major store.
"""

import numpy as np

B, C, S = 64, 256, 4096
NCORES = 8
BPC = B // NCORES          # batches per core
NJ = S // 512              # 8 psum-bank chunks of 512
H = C // 128               # 2 channel halves

_cache = {}


def _build_nc(fold_batches=2, use_f32r=False):
    import concourse.bacc as bacc
    import concourse.bass as bass
    import concourse.mybir as mybir
    from concourse.tile import TileContext

    f32 = mybir.dt.float32
    Alu = mybir.AluOpType

    nc = bacc.Bacc(
        "TRN2",
        target_bir_lowering=False,
        debug=False,
        num_devices=NCORES,
    )

    x_d = nc.dram_tensor("x", [BPC, C, S], f32, kind="ExternalInput").ap()
    w_d = nc.dram_tensor("weights", [C, S], f32, kind="ExternalInput").ap()
    b_d = nc.dram_tensor("bias", [S], f32, kind="ExternalInput").ap()
    o_d = nc.dram_tensor("out", [BPC, S], f32, kind="ExternalOutput").ap()

    with TileContext(nc) as tc:
        NQ = 4                  # s-quarters per c-half for DMA/compute chunking
        QS = S // NQ
        with (
            tc.tile_pool(name="consts", bufs=1) as cpool,
            tc.tile_pool(name="xbuf", bufs=3) as xpool,
            tc.tile_pool(name="stg", bufs=2) as spool,
            tc.tile_pool(name="epi", bufs=1) as epool,
            tc.tile_pool(name="ps", bufs=1, space="PSUM") as ppool,
        ):
            # Replicated weights, both halves side by side: [:, h*S:(h+1)*S].
            # W loads are emitted per (h, q) chunk, interleaved with batch 0's
            # x chunks below, so the first multiply starts after ~2 MiB of
            # DMA instead of waiting for all of W.
            w_t = cpool.tile([128, H * S], f32, name="w_t")

            ones_t = cpool.tile([128, 1], f32, name="ones_t")
            nc.vector.memset(ones_t[:], 1.0)

            # PE output rows must sit on 32-aligned partitions, and a PSUM
            # bank being read (ACT drain) while the PE writes it serializes
            # the pipeline. Slot map: batch parity picks the bank half
            # (free-dim half), (b//2)%2 picks the row pair — consecutive
            # batches touch disjoint banks, so drains overlap next batch's
            # matmuls. Each batch's 4096-wide row lives as 2 half-rows:
            #   chunk j -> row 32*(2*((b//2)%2) + j//4),
            #             free offset (S//2)*(b%2) + (j%4)*512.
            psum_big = ppool.tile([128, S], f32, name="psum_big")
            # out_acc is pre-loaded with bias; each batch's sums are packed
            # onto row b with an ACCUMULATING SWDGE DMA (out += stg), which
            # fuses the bias add for free. Small/late-bound DMAs (bias,
            # drain-pack, stores) go on the scalar/gpsimd queues so they
            # can't head-of-line-block the x prefetch stream on sync.
            out_acc = epool.tile([BPC, S], f32, name="out_acc")
            for bb in range(BPC):
                nc.scalar.dma_start(out_acc[bb:bb + 1, :], b_d[None, :])

            def chunk(base, h, q):
                return slice(base + h * S + q * QS, base + h * S + (q + 1) * QS)

            for b in range(BPC):
                hb = b % 2              # bank half (free-dim half)
                rp = (b // 2) % 2       # row pair
                # Separate tile per c-half: a batch's h1 tile frees as soon
                # as its fold/matmuls are done, giving finer slot recycling
                # than one double-width tile.
                xh = [
                    xpool.tile([128, S], f32, name=f"x_h{h}", tag=f"x{h}", bufs=3)
                    for h in range(H)
                ]
                # 2 MiB DMA transfers (best bandwidth); DVE still computes in
                # QS-wide chunks for pipelining.
                for h in range(H):
                    for dq in range(2):
                        lo, hi = dq * (S // 2), (dq + 1) * (S // 2)
                        if b == 0:
                            nc.sync.dma_start(
                                w_t[:, h * S + lo:h * S + hi],
                                w_d[h * 128:(h + 1) * 128, lo:hi],
                            )
                        nc.sync.dma_start(
                            xh[h][:, lo:hi],
                            x_d[b, h * 128:(h + 1) * 128, lo:hi],
                        )
                fold = b >= BPC - fold_batches
                nhalf = 1 if fold else H
                for q in range(NQ):
                    qs = slice(q * QS, (q + 1) * QS)
                    for h in range(H):
                        nc.vector.tensor_tensor(
                            xh[h][:, qs], xh[h][:, qs], w_t[:, chunk(0, h, q)],
                            Alu.mult,
                        )
                    if fold:
                        # z = y_h0 + y_h1 in place -> halves the PE stream
                        nc.vector.tensor_tensor(
                            xh[0][:, qs], xh[0][:, qs], xh[1][:, qs], Alu.add
                        )
                    for j in (2 * q, 2 * q + 1):
                        row = 32 * (2 * rp + j // 4)
                        off = (S // 2) * hb + (j % 4) * 512
                        for h in range(nhalf):
                            rhs = xh[h][:, j * 512:(j + 1) * 512]
                            lhsT = ones_t[:, 0:1]
                            if use_f32r:
                                rhs = rhs.bitcast(mybir.dt.float32r)
                                lhsT = lhsT.bitcast(mybir.dt.float32r)
                            nc.tensor.matmul(
                                psum_big[row:row + 1, off:off + 512],
                                lhsT,
                                rhs,
                                start=(h == 0),
                                stop=(h == nhalf - 1),
                                tile_position=(0, row),
                            )
                # Drain this batch's two half-rows: compute engines can only
                # address 32-aligned SBUF partition windows, so ACT-copy each
                # psum half-row to a partition-0 staging row, then pack it
                # onto partition b of out_acc with an SBUF->SBUF DMA
                # (DMA has no partition-alignment restriction).
                stg = spool.tile([1, S], f32, name="stg", tag="stg")
                for half in range(2):
                    row = 32 * (2 * rp + half)
                    off = (S // 2) * hb
                    nc.scalar.activation(
                        stg[:, half * (S // 2):(half + 1) * (S // 2)],
                        psum_big[row:row + 1, off:off + S // 2],
                        mybir.ActivationFunctionType.Copy,
                    )
                nc.gpsimd.dma_start(
                    out_acc[b:b + 1, :], stg[:, :], accum_op=Alu.add
                )

            # Epilogue: capped relu on [8, 4096] in two s-halves, computed
            # in place on out_acc (bias already folded in by the accumulating
            # pack DMAs), then row-major store.
            for s0 in (0, S // 2):
                sl = slice(s0, s0 + S // 2)
                msk = epool.tile([BPC, S // 2], f32, name="msk", tag="msk", bufs=1)
                nc.vector.tensor_scalar(msk[:], out_acc[:, sl], 0.0, 1.0, Alu.max, Alu.is_le)
                nc.vector.scalar_tensor_tensor(
                    out_acc[:, sl], out_acc[:, sl], 0.0, msk[:], Alu.max, Alu.mult
                )
                nc.scalar.dma_start(o_d[:, sl], out_acc[:, sl])

    nc.compile()
    return nc


def kernel(x: np.ndarray, weights: np.ndarray, bias: np.ndarray) -> np.ndarray:
    from concourse.bass_utils import run_bass_kernel_spmd

    if "nc" not in _cache:
        _cache["nc"] = _build_nc()
    nc = _cache["nc"]

    x = np.ascontiguousarray(x, dtype=np.float32)
    weights = np.ascontiguousarray(weights, dtype=np.float32)
    bias = np.ascontiguousarray(bias, dtype=np.float32)

    in_maps = [
        {
            "x": x[i * BPC:(i + 1) * BPC],
            "weights": weights,
            "bias": bias,
        }
        for i in range(NCORES)
    ]
    res = run_bass_kernel_spmd(nc, in_maps, core_ids=list(range(NCORES)))
    return np.concatenate([res.results[i]["out"] for i in range(NCORES)], axis=0)



# revision 6
# speedup vs baseline: 1.1832x; 1.1832x over previous
"""Trainium2 Bass kernel for channel-wise weighted reduction + capped relu.

Computes out[b, s] = capped_relu(sum_c x[b,c,s] * W[c,s] + bias[s]) for
x [64, 256, 4096] f32, W [256, 4096] f32, bias [4096] f32.

Sharding: S-parallel across 8 NeuronCores — core k owns s-columns
[512k, 512(k+1)) for ALL 64 batches.  Per-core HBM traffic: x 32 MiB +
W 0.5 MiB (vs 4 MiB replicated under batch sharding) + out 128 KiB, so
the DMA floor drops from ~105 us to ~96 us at the 360 GB/s per-core
aggregate.  No cross-core communication.

Per-core pipeline (64 batches as 16 groups of 4; free dim = 4 b x 512 s):
  - DMA x group-half as one SBUF tile [128ch, 4*512] (1 MiB, 2 KiB rows).
  - DVE: y_h = x_h * W_h with W read through a stride-0 broadcast AP
    (one [128,512] W tile serves all 4 batches); y tiles are dtype
    float32r — the DVE rounds, which the BIR verifier requires for f32r
    matmul inputs.
  - PE:  channel reduction as matmul, ones[128,1] f32r stationary, y
    moving.  f32r streams 1 row/cycle vs fp32's 4 => ~4x less PE busy.
    f32r matmuls may only write psum partition 0, so a group's 4x512
    outputs live on row 0, alternating bank halves (offset 2048*(g%2))
    between consecutive groups so drains overlap next group's matmuls.
    The two c-halves accumulate via start/stop into the same psum slot.
  - Drain: ACT-copy the group's psum half-row to a staging row, then
    accumulate onto out_acc[g] with an SWDGE accum DMA.  out_acc is
    preloaded with the replicated bias (host-tiled [16, 2048] input),
    fusing the bias add.
  - Epilogue on [16, 2048]: tb = out_acc ; mask = is_le(max(tb,0),1) ;
    o = max(tb,0)*mask ; store to the out[64, 512] shard.
"""

import numpy as np

B, C, S = 64, 256, 4096
NCORES = 8
SS = S // NCORES           # s-columns per core (512)
GB = 4                     # batches per group
G = B // GB                # batch groups per core (16)
H = C // 128               # channel halves
FREE = GB * SS             # free width of packed tiles (2048)

_cache = {}


def _build_nc():
    import concourse.bacc as bacc
    import concourse.mybir as mybir
    from concourse.tile import TileContext

    f32 = mybir.dt.float32
    f32r = mybir.dt.float32r
    Alu = mybir.AluOpType

    nc = bacc.Bacc(
        "TRN2",
        target_bir_lowering=False,
        debug=False,
        num_devices=NCORES,
    )

    x_d = nc.dram_tensor("x", [B, C, SS], f32, kind="ExternalInput").ap()
    w_d = nc.dram_tensor("weights", [C, SS], f32, kind="ExternalInput").ap()
    b_d = nc.dram_tensor("bias_rep", [G, FREE], f32, kind="ExternalInput").ap()
    o_d = nc.dram_tensor("out", [B, SS], f32, kind="ExternalOutput").ap()

    with TileContext(nc) as tc:
        with (
            tc.tile_pool(name="consts", bufs=1) as cpool,
            tc.tile_pool(name="xbuf", bufs=5) as xpool,
            tc.tile_pool(name="ybuf", bufs=2) as ypool,
            tc.tile_pool(name="stg", bufs=2) as spool,
            tc.tile_pool(name="epi", bufs=1) as epool,
            tc.tile_pool(name="ps", bufs=1, space="PSUM") as ppool,
        ):
            # W halves, loaded once (0.5 MiB total).
            w_t = cpool.tile([128, H * SS], f32, name="w_t")
            for h in range(H):
                nc.scalar.dma_start(
                    w_t[:, h * SS:(h + 1) * SS], w_d[h * 128:(h + 1) * 128, :]
                )

            # memset can't emit float32r; round 1.0f through a DVE ALU op.
            ones_f = cpool.tile([128, 1], f32, name="ones_f")
            nc.vector.memset(ones_f[:], 1.0)
            ones_t = cpool.tile([128, 1], f32r, name="ones_t")
            nc.vector.tensor_scalar_add(ones_t[:], ones_f[:], 0.0)

            psum_big = ppool.tile([128, S], f32, name="psum_big")
            # out_acc preloaded with host-replicated bias; group sums are
            # packed onto row g with accumulating SWDGE DMAs.
            out_acc = epool.tile([G, FREE], f32, name="out_acc")
            nc.scalar.dma_start(out_acc[:, :], b_d[:, :])

            for g in range(G):
                off0 = (g % 2) * FREE   # psum row-0 bank half
                xh = [
                    xpool.tile([128, FREE], f32, name=f"x_h{h}", tag=f"x{h}", bufs=5)
                    for h in range(H)
                ]
                yh = [
                    ypool.tile([128, FREE], f32r, name=f"y_h{h}", tag=f"y{h}", bufs=2)
                    for h in range(H)
                ]
                for h in range(H):
                    # [128 ch, 4 b, 512 s] gather: 512 descriptors x 2 KiB.
                    nc.sync.dma_start(
                        xh[h][:, :],
                        x_d[g * GB:(g + 1) * GB, h * 128:(h + 1) * 128, :]
                        .transpose([1, 0, 2]),
                    )
                for h in range(H):
                    # One [128, 2048] multiply per half; W chunk broadcast
                    # along the batch axis via a stride-0 AP.
                    w_b = (
                        w_t[:, h * SS:(h + 1) * SS]
                        .unsqueeze(1)
                        .broadcast_to([128, GB, SS])
                    )
                    nc.vector.tensor_tensor(
                        yh[h][:, :].rearrange("p (b s) -> p b s", b=GB),
                        xh[h][:, :].rearrange("p (b s) -> p b s", b=GB),
                        w_b,
                        Alu.mult,
                    )
                for j in range(GB):
                    off = off0 + j * 512
                    for h in range(H):
                        nc.tensor.matmul(
                            psum_big[0:1, off:off + 512],
                            ones_t[:, 0:1],
                            yh[h][:, j * 512:(j + 1) * 512],
                            start=(h == 0),
                            stop=(h == H - 1),
                        )
                # Drain the group's psum half-row via ACT to a partition-0
                # staging row, then accumulate onto out_acc[g] (bias there).
                stg = spool.tile([1, FREE], f32, name="stg", tag="stg")
                nc.scalar.activation(
                    stg[:, :],
                    psum_big[0:1, off0:off0 + FREE],
                    mybir.ActivationFunctionType.Copy,
                )
                nc.gpsimd.dma_start(
                    out_acc[g:g + 1, :], stg[:, :], accum_op=Alu.add
                )

            # Epilogue: capped relu in two free-halves, then store.  Free
            # half fh covers batch-within-group b4 in [2*fh, 2*fh+2), i.e.
            # out rows 4g + b4.
            for fh in range(2):
                sl = slice(fh * (FREE // 2), (fh + 1) * (FREE // 2))
                msk = epool.tile([G, FREE // 2], f32, name="msk", tag="msk", bufs=1)
                nc.vector.tensor_scalar(
                    msk[:], out_acc[:, sl], 0.0, 1.0, Alu.max, Alu.is_le
                )
                nc.vector.scalar_tensor_tensor(
                    out_acc[:, sl], out_acc[:, sl], 0.0, msk[:], Alu.max, Alu.mult
                )
                # dest rows b = 4g + b4, b4 in [2fh, 2fh+2)
                dst = (
                    o_d.rearrange("(g b) s -> g b s", g=G)[:, 2 * fh:2 * fh + 2, :]
                )
                nc.scalar.dma_start(
                    dst,
                    out_acc[:, sl].rearrange("p (b s) -> p b s", b=GB // 2),
                )

    nc.compile()
    return nc


def shard_inputs(x, weights, bias):
    """Per-core input shards for S-parallel layout."""
    x = np.ascontiguousarray(x, dtype=np.float32)
    weights = np.ascontiguousarray(weights, dtype=np.float32)
    bias = np.ascontiguousarray(bias, dtype=np.float32)
    maps = []
    for k in range(NCORES):
        sl = slice(k * SS, (k + 1) * SS)
        maps.append(
            {
                "x": np.ascontiguousarray(x[:, :, sl]),
                "weights": np.ascontiguousarray(weights[:, sl]),
                "bias_rep": np.tile(bias[sl], (G, GB)).astype(np.float32),
            }
        )
    return maps


def kernel(x: np.ndarray, weights: np.ndarray, bias: np.ndarray) -> np.ndarray:
    from concourse.bass_utils import run_bass_kernel_spmd

    if "nc" not in _cache:
        _cache["nc"] = _build_nc()
    nc = _cache["nc"]

    in_maps = shard_inputs(x, weights, bias)
    res = run_bass_kernel_spmd(nc, in_maps, core_ids=list(range(NCORES)))
    out = np.empty((B, S), dtype=np.float32)
    for k in range(NCORES):
        out[:, k * SS:(k + 1) * SS] = res.results[k]["out"]
    return out


# revision 8
# speedup vs baseline: 1.2658x; 1.0698x over previous
"""Trainium2 Bass kernel for channel-wise weighted reduction + capped relu.

Computes out[b, s] = capped_relu(sum_c x[b,c,s] * W[c,s] + bias[s]) for
x [64, 256, 4096] f32, W [256, 4096] f32, bias [4096] f32.

Sharding: S-parallel across 8 NeuronCores — core k owns s-columns
[512k, 512(k+1)) for ALL 64 batches.  Per-core HBM traffic: x 32 MiB +
W 0.5 MiB (vs 4 MiB replicated under batch sharding) + out 128 KiB, so
the DMA floor drops from ~105 us to ~96 us at the 360 GB/s per-core
aggregate.  No cross-core communication.

Per-core pipeline (64 batches as 16 groups of 4; free dim = 4 b x 512 s):
  - DMA x group-half as one SBUF tile [128ch, 4*512] (1 MiB, 2 KiB rows).
  - DVE: y_h = x_h * W_h with W read through a stride-0 broadcast AP
    (one [128,512] W tile serves all 4 batches); y tiles are dtype
    float32r — the DVE rounds, which the BIR verifier requires for f32r
    matmul inputs.
  - PE:  channel reduction as matmul, ones[128,1] f32r stationary, y
    moving.  f32r streams 1 row/cycle vs fp32's 4 => ~4x less PE busy.
    f32r matmuls may only write psum partition 0, so a group's 4x512
    outputs live on row 0, alternating bank halves (offset 2048*(g%2))
    between consecutive groups so drains overlap next group's matmuls.
    The two c-halves accumulate via start/stop into the same psum slot.
  - Drain: ACT-copy the group's psum half-row to a staging row, then
    accumulate onto out_acc[g] with an SWDGE accum DMA.  out_acc is
    preloaded with the replicated bias (host-tiled [16, 2048] input),
    fusing the bias add.
  - Epilogue on [16, 2048]: tb = out_acc ; mask = is_le(max(tb,0),1) ;
    o = max(tb,0)*mask ; store to the out[64, 512] shard.
"""

import numpy as np

B, C, S = 64, 256, 4096
NCORES = 8
SS = S // NCORES           # s-columns per core (512)
GB = 4                     # batches per group
G = B // GB                # batch groups per core (16)
H = C // 128               # channel halves
FREE = GB * SS             # free width of packed tiles (2048)

_cache = {}


def _build_nc():
    import concourse.bacc as bacc
    import concourse.mybir as mybir
    from concourse.tile import TileContext

    f32 = mybir.dt.float32
    f32r = mybir.dt.float32r
    Alu = mybir.AluOpType

    nc = bacc.Bacc(
        "TRN2",
        target_bir_lowering=False,
        debug=False,
        num_devices=NCORES,
    )

    x_d = nc.dram_tensor("x", [B, C, SS], f32, kind="ExternalInput").ap()
    w_d = nc.dram_tensor("weights", [C, SS], f32, kind="ExternalInput").ap()
    b_d = nc.dram_tensor("bias_rep", [G, FREE], f32, kind="ExternalInput").ap()
    o_d = nc.dram_tensor("out", [B, SS], f32, kind="ExternalOutput").ap()

    with TileContext(nc) as tc:
        with (
            tc.tile_pool(name="consts", bufs=1) as cpool,
            tc.tile_pool(name="xbuf", bufs=5) as xpool,
            tc.tile_pool(name="ybuf", bufs=2) as ypool,
            tc.tile_pool(name="stg", bufs=2) as spool,
            tc.tile_pool(name="epi", bufs=1) as epool,
            tc.tile_pool(name="ps", bufs=1, space="PSUM") as ppool,
        ):
            # W halves, loaded once (0.5 MiB total).
            w_t = cpool.tile([128, H * SS], f32, name="w_t")
            for h in range(H):
                nc.scalar.dma_start(
                    w_t[:, h * SS:(h + 1) * SS], w_d[h * 128:(h + 1) * 128, :]
                )

            # memset can't emit float32r; round 1.0f through a DVE ALU op.
            ones_f = cpool.tile([128, 1], f32, name="ones_f")
            nc.vector.memset(ones_f[:], 1.0)
            ones_t = cpool.tile([128, 1], f32r, name="ones_t")
            nc.vector.tensor_scalar_add(ones_t[:], ones_f[:], 0.0)

            psum_big = ppool.tile([128, S], f32, name="psum_big")
            # out_acc preloaded with host-replicated bias; group sums are
            # packed onto row g with accumulating SWDGE DMAs.
            out_acc = epool.tile([G, FREE], f32, name="out_acc")
            nc.scalar.dma_start(out_acc[:, :], b_d[:, :])

            # Two groups per DMA (2 MiB transfers) halves the per-transfer
            # queue bubble (post-DMA semaphore latency).  Only SP and ACT
            # have HWDGE queues, and ACT runs the psum drains, so the whole
            # x stream stays on sync.
            qeng = [nc.sync, nc.sync]
            for p in range(G // 2):
                xh = [
                    xpool.tile([128, 2 * FREE], f32, name=f"x_h{h}", tag=f"x{h}", bufs=3)
                    for h in range(H)
                ]
                for h in range(H):
                    # [128 ch, 8 b, 512 s] gather: 1024 descriptors x 2 KiB.
                    qeng[h].dma_start(
                        xh[h][:, :],
                        x_d[p * 2 * GB:(p + 1) * 2 * GB, h * 128:(h + 1) * 128, :]
                        .transpose([1, 0, 2]),
                    )
                for sub in range(2):
                    g = 2 * p + sub
                    off0 = (g % 2) * FREE   # psum row-0 bank half
                    yh = [
                        ypool.tile([128, FREE], f32r, name=f"y_h{h}", tag=f"y{h}", bufs=2)
                        for h in range(H)
                    ]
                    for h in range(H):
                        # One [128, 2048] multiply per half; W chunk broadcast
                        # along the batch axis via a stride-0 AP.
                        w_b = (
                            w_t[:, h * SS:(h + 1) * SS]
                            .unsqueeze(1)
                            .broadcast_to([128, GB, SS])
                        )
                        nc.vector.tensor_tensor(
                            yh[h][:, :].rearrange("p (b s) -> p b s", b=GB),
                            xh[h][:, sub * FREE:(sub + 1) * FREE]
                            .rearrange("p (b s) -> p b s", b=GB),
                            w_b,
                            Alu.mult,
                        )
                    for j in range(GB):
                        off = off0 + j * 512
                        for h in range(H):
                            nc.tensor.matmul(
                                psum_big[0:1, off:off + 512],
                                ones_t[:, 0:1],
                                yh[h][:, j * 512:(j + 1) * 512],
                                start=(h == 0),
                                stop=(h == H - 1),
                            )
                    # Drain the group's psum half-row via ACT to a partition-0
                    # staging row, then accumulate onto out_acc[g].
                    stg = spool.tile([1, FREE], f32, name="stg", tag="stg")
                    nc.scalar.activation(
                        stg[:, :],
                        psum_big[0:1, off0:off0 + FREE],
                        mybir.ActivationFunctionType.Copy,
                    )
                    nc.gpsimd.dma_start(
                        out_acc[g:g + 1, :], stg[:, :], accum_op=Alu.add
                    )

            # Epilogue: capped relu in two free-halves, then store.  Free
            # half fh covers batch-within-group b4 in [2*fh, 2*fh+2), i.e.
            # out rows 4g + b4.
            for fh in range(2):
                sl = slice(fh * (FREE // 2), (fh + 1) * (FREE // 2))
                msk = epool.tile([G, FREE // 2], f32, name="msk", tag="msk", bufs=1)
                nc.vector.tensor_scalar(
                    msk[:], out_acc[:, sl], 0.0, 1.0, Alu.max, Alu.is_le
                )
                nc.vector.scalar_tensor_tensor(
                    out_acc[:, sl], out_acc[:, sl], 0.0, msk[:], Alu.max, Alu.mult
                )
                # dest rows b = 4g + b4, b4 in [2fh, 2fh+2)
                dst = (
                    o_d.rearrange("(g b) s -> g b s", g=G)[:, 2 * fh:2 * fh + 2, :]
                )
                nc.scalar.dma_start(
                    dst,
                    out_acc[:, sl].rearrange("p (b s) -> p b s", b=GB // 2),
                )

    nc.compile()
    return nc


def shard_inputs(x, weights, bias):
    """Per-core input shards for S-parallel layout."""
    x = np.ascontiguousarray(x, dtype=np.float32)
    weights = np.ascontiguousarray(weights, dtype=np.float32)
    bias = np.ascontiguousarray(bias, dtype=np.float32)
    maps = []
    for k in range(NCORES):
        sl = slice(k * SS, (k + 1) * SS)
        maps.append(
            {
                "x": np.ascontiguousarray(x[:, :, sl]),
                "weights": np.ascontiguousarray(weights[:, sl]),
                "bias_rep": np.tile(bias[sl], (G, GB)).astype(np.float32),
            }
        )
    return maps


def kernel(x: np.ndarray, weights: np.ndarray, bias: np.ndarray) -> np.ndarray:
    from concourse.bass_utils import run_bass_kernel_spmd

    if "nc" not in _cache:
        _cache["nc"] = _build_nc()
    nc = _cache["nc"]

    in_maps = shard_inputs(x, weights, bias)
    res = run_bass_kernel_spmd(nc, in_maps, core_ids=list(range(NCORES)))
    out = np.empty((B, S), dtype=np.float32)
    for k in range(NCORES):
        out[:, k * SS:(k + 1) * SS] = res.results[k]["out"]
    return out
